# revision 1
# baseline (speedup 1.0000x reference)
"""Trainium2 Bass kernel for nn_CanineAttention (block-diagonal local attention).

Reference computation (per batch b):
  q/k/v = x @ W{q,k,v} + b{q,k,v}            x: [B,S,H]=[4,4096,768]
  per chunk of W=128 tokens, per head (NH=12, HD=64):
    scores = q k^T / 8 + (1-mask_diag)*(-1e4); probs = softmax(scores)
    ctx = probs @ v
  out = LayerNorm(ctx @ Wo + bo + x) * gamma + beta

Sharding: tokens (B*S = 16384) are split contiguously across 8 cores
(2048 tokens = 16 chunks per core; chunk boundaries align), fully
data-parallel, no collectives.

Device layout notes:
  - Projections run with the contraction dim (h_in) on partitions, so the
    kernel consumes x TRANSPOSED (host pre-transposes to [H, tokens]).
  - qT, kT are produced in [h_out, t] layout (head dims on partitions),
    v in natural [t, h_out] layout; attention produces ctxT [h_out, t]
    which feeds the output projection as lhsT directly.
  - Softmax skips max-subtraction (scores are O(1) here; the additive mask
    bias is <= 0 so exp() only underflows, never overflows).
  - Precision modes:
      "f32"    : native fp32 matmuls (4 cycles/row on PE)
      "bf16"   : single bf16 pass (1 cycle/row)
      "split3" : x=hi+lo, W=hi+lo in bf16; q = xh@Wh + xh@Wl + xl@Wh
                 (3 cycles/row, ~fp32 accuracy)
    Attention internals (scores/softmax/PV) are always fp32.
"""

import numpy as np
import ml_dtypes
from contextlib import ExitStack

import concourse.bass as bass
import concourse.tile as tile
from concourse import bacc, mybir
from concourse.bass_utils import run_bass_kernel_spmd
from concourse.masks import make_identity

# ---------------- problem constants (hardcoded per contract) ----------------
B, S, H, NH, W = 4, 4096, 768, 12, 128
HD = H // NH            # 64
C = S // W              # 32 chunks
NEG = -10000.0
EPS = 1e-12

NCORES = 8
TPC = B * S // NCORES   # 2048 tokens per core
CPC = TPC // W          # 16 chunks per core
BLK = 512               # tokens per processing block
NBLK = TPC // BLK       # 4 blocks
CPB = BLK // W          # 4 chunks per block
NG = H // 128           # 6 partition-chunks over H
NPAIR = NH // 2         # 6 head pairs (2 heads x 64 dims = 128 partitions)

F32 = mybir.dt.float32
BF16 = mybir.dt.bfloat16
FP = mybir.ActivationFunctionType
OP = mybir.AluOpType
AX = mybir.AxisListType

MODE = "split3"   # default precision mode; see module docstring

F16 = mybir.dt.float16
MODES = {
    # dt: matmul operand dtype; xsplit: x = hi+lo; wsplit: W = hi+lo
    "f32":    dict(dt=F32,  xsplit=False, wsplit=False),
    "bf16":   dict(dt=BF16, xsplit=False, wsplit=False),
    "fp16":   dict(dt=F16,  xsplit=False, wsplit=False),
    "fp16s2": dict(dt=F16,  xsplit=False, wsplit=True),
    "split3": dict(dt=BF16, xsplit=True,  wsplit=True),
    "fp16s3": dict(dt=F16,  xsplit=True,  wsplit=True),
}


def _bf16(a):
    return a.astype(ml_dtypes.bfloat16)


# ---------------------------------------------------------------------------
# kernel builder
# ---------------------------------------------------------------------------

def _build(mode, use_mask, use_qbias, use_kbias, use_vbias, use_ln_affine, reps=1):
    """Build + compile the SPMD single-core program. Returns (nc, input_names).

    reps>1 repeats the whole computation (idempotent output writes) so HW
    kernel time can be measured as the slope over reps."""
    nc = bacc.Bacc(
        "TRN2", target_bir_lowering=False, debug=False,
        enable_asserts=False, num_devices=NCORES,
    )

    xsplit, wsplit = MODES[mode]["xsplit"], MODES[mode]["wsplit"]
    split = xsplit
    wdt = MODES[mode]["dt"]
    xdt = wdt
    # attention matmul operand dtype: fp16 internals cost ~2e-6 extra e2e
    # error in fp16 mode but run the PE at 1 cycle/row instead of 4
    adt = F16 if mode == "fp16" else F32

    # ---------------- DRAM I/O ----------------
    names = []

    def dram_in(name, shape, dt):
        names.append(name)
        return nc.dram_tensor(name, shape, dt, kind="ExternalInput").ap()

    xt_hi = dram_in("xt_hi", [H, TPC], xdt)            # x^T (hi part if split)
    xt_lo = dram_in("xt_lo", [H, TPC], xdt) if xsplit else None
    xres = dram_in("xres", [TPC, H], F32)              # x + bo (residual input)
    w_dram = {"wq": dram_in("wq_hi", [H, H], wdt),
              "wk": dram_in("wk_hi", [H, H], wdt),
              "wv": dram_in("wv_hi", [H, H], wdt),
              "wo": dram_in("wo_hi", [H, H], wdt)}
    if wsplit:
        for wn in ("wq", "wk", "wv", "wo"):
            w_dram[wn + "_lo"] = dram_in(wn + "_lo", [H, H], wdt)
    bq = dram_in("bq", [128, NG], F32) if use_qbias else None   # (bq/8) chunked
    bk = dram_in("bk", [128, NG], F32) if use_kbias else None
    bvb = dram_in("bvb", [128, H], F32) if use_vbias else None  # bv broadcast
    gmb = dram_in("gmb", [128, H], F32) if use_ln_affine else None
    btb = dram_in("btb", [128, H], F32) if use_ln_affine else None
    mbias = dram_in("mbias", [CPC, W, W], F32) if use_mask else None
    out = nc.dram_tensor("out", [TPC, H], F32, kind="ExternalOutput").ap()

    # matmul pass list: (x-half, weight-key)
    if xsplit and wsplit:
        passes = {wn: [("hi", wn), ("hi", wn + "_lo"), ("lo", wn)]
                  for wn in ("wq", "wk", "wv", "wo")}
    elif wsplit:
        passes = {wn: [("hi", wn), ("hi", wn + "_lo")]
                  for wn in ("wq", "wk", "wv", "wo")}
    else:
        passes = {wn: [("hi", wn)] for wn in ("wq", "wk", "wv", "wo")}

    with tile.TileContext(nc) as tc, ExitStack() as ctx:
        const = ctx.enter_context(tc.tile_pool(name="const", bufs=1))
        xp = ctx.enter_context(tc.tile_pool(name="xp", bufs=2))
        qkv = ctx.enter_context(tc.tile_pool(name="qkv", bufs=1))
        attn = ctx.enter_context(tc.tile_pool(name="attn", bufs=3))
        ctxp = ctx.enter_context(tc.tile_pool(name="ctxp", bufs=1))
        outp = ctx.enter_context(tc.tile_pool(name="outp", bufs=2))
        pproj = ctx.enter_context(tc.tile_pool(name="pproj", bufs=2, space="PSUM"))
        psc = ctx.enter_context(tc.tile_pool(name="psc", bufs=2, space="PSUM"))
        ppt = ctx.enter_context(tc.tile_pool(name="ppt", bufs=2, space="PSUM"))
        pcx = ctx.enter_context(tc.tile_pool(name="pcx", bufs=2, space="PSUM"))

        # ---------------- constants ----------------
        ident = const.tile([128, 128], adt, tag="ident")
        make_identity(nc, ident)

        w_sb = {}   # key -> list of NG chunk tiles [128, H]
        for wn, ap in w_dram.items():
            w_sb[wn] = []
            for g in range(NG):
                t = const.tile([128, H], wdt, tag=f"{wn}{g}")
                nc.sync.dma_start(t[:], ap[g * 128:(g + 1) * 128, :])
                w_sb[wn].append(t)

        bq_sb = bk_sb = bvb_sb = gmb_sb = btb_sb = None
        if use_qbias:
            bq_sb = const.tile([128, NG], F32, tag="bq")
            nc.sync.dma_start(bq_sb[:], bq)
        if use_kbias:
            bk_sb = const.tile([128, NG], F32, tag="bk")
            nc.sync.dma_start(bk_sb[:], bk)
        if use_vbias:
            bvb_sb = const.tile([128, H], F32, tag="bvb")
            nc.sync.dma_start(bvb_sb[:], bvb)
        if use_ln_affine:
            gmb_sb = const.tile([128, H], F32, tag="gmb")
            nc.sync.dma_start(gmb_sb[:], gmb)
            btb_sb = const.tile([128, H], F32, tag="btb")
            nc.sync.dma_start(btb_sb[:], btb)

        # ---------------- per token-block ----------------
        # reps>1: device-side hardware loop repeating the whole computation
        # (for slope-based HW timing); body is identical each iteration.
        import contextlib
        rep_cm = tc.For_i(0, reps, 1) if reps > 1 else contextlib.nullcontext()
        with rep_cm:
          for blk in range(NBLK):
            t0 = blk * BLK

            # -- load x^T block --
            xth = [xp.tile([128, BLK], xdt, tag=f"xth{g}", name=f"xth{g}") for g in range(NG)]
            for g in range(NG):
                nc.sync.dma_start(xth[g][:], xt_hi[g * 128:(g + 1) * 128, t0:t0 + BLK])
            if xsplit:
                xtl = [xp.tile([128, BLK], xdt, tag=f"xtl{g}", name=f"xtl{g}") for g in range(NG)]
                for g in range(NG):
                    nc.sync.dma_start(xtl[g][:], xt_lo[g * 128:(g + 1) * 128, t0:t0 + BLK])

            def xop(sel, g):
                return xth[g] if sel == "hi" else xtl[g]

            # -- Q/K projections (transposed layout [h_out, t]) --
            qT, kT = [], []
            for which, wn, dst in (("q", "wq", qT), ("k", "wk", kT)):
                for go in range(NG):
                    ps = pproj.tile([128, BLK], F32, tag="proj")
                    mms = [(wkey, gi, xsel) for xsel, wkey in passes[wn] for gi in range(NG)]
                    for i, (wkey, gi, xsel) in enumerate(mms):
                        nc.tensor.matmul(
                            ps[:],
                            w_sb[wkey][gi][:, go * 128:(go + 1) * 128],
                            xop(xsel, gi)[:],
                            start=(i == 0), stop=(i == len(mms) - 1),
                        )
                    sb = qkv.tile([128, BLK], adt, tag=f"{which}T{go}")
                    scl = 0.125 if which == "q" else 1.0
                    has_b = use_qbias if which == "q" else use_kbias
                    if has_b:
                        bias = (bq_sb if which == "q" else bk_sb)[:, go:go + 1]
                        nc.scalar.activation(sb[:], ps[:], FP.Identity, bias=bias, scale=scl)
                    else:
                        nc.scalar.activation(sb[:], ps[:], FP.Copy, scale=scl)
                    dst.append(sb)

            # -- V projection (natural layout [t, h_out]) --
            vN = []
            for tt in range(CPB):
                vt = qkv.tile([128, H], adt, tag=f"v{tt}")
                for nhalf in range(2):
                    n0 = nhalf * 384
                    ps = pproj.tile([128, 384], F32, tag="proj")
                    mms = [(wkey, gi, xsel) for xsel, wkey in passes["wv"] for gi in range(NG)]
                    for i, (wkey, gi, xsel) in enumerate(mms):
                        nc.tensor.matmul(
                            ps[:],
                            xop(xsel, gi)[:, tt * 128:(tt + 1) * 128],
                            w_sb[wkey][gi][:, n0:n0 + 384],
                            start=(i == 0), stop=(i == len(mms) - 1),
                        )
                    if use_vbias:
                        nc.vector.tensor_add(vt[:, n0:n0 + 384], ps[:], bvb_sb[:, n0:n0 + 384])
                    else:
                        nc.vector.tensor_copy(vt[:, n0:n0 + 384], ps[:])
                vN.append(vt)

            # -- block-diagonal attention --
            # ctxT tiles [128, BLK] per partition-chunk (= head pair)
            cxdt = F32 if mode == "f32" else wdt
            cxh = [ctxp.tile([128, BLK], cxdt, tag=f"cxh{g}", name=f"cxh{g}") for g in range(NG)]
            cxl = [ctxp.tile([128, BLK], wdt, tag=f"cxl{g}", name=f"cxl{g}") for g in range(NG)] if wsplit else None

            for cc in range(CPB):
                ts = slice(cc * 128, (cc + 1) * 128)
                chunk_idx = blk * CPB + cc
                if use_mask:
                    mb = attn.tile([128, W], F32, tag="mb")
                    nc.sync.dma_start(mb[:], mbias[chunk_idx])
                for g in range(NPAIR):
                    # scores for head pair (2g, 2g+1); row-tiled concurrent
                    # matmuls MUST land in separate PSUM banks (same-bank
                    # concurrent writes are a HW collision).
                    ps_s = [psc.tile([128, W], F32, tag="sc", name=f"scps{h}")
                            for h in range(2)]
                    for h in range(2):
                        p0 = h * 64
                        nc.tensor.matmul(
                            ps_s[h][:],
                            qT[g][p0:p0 + 64, ts],
                            kT[g][p0:p0 + 64, ts],
                            start=True, stop=True,
                            tile_position=(p0, 0),
                        )
                    den = attn.tile([128, 2], F32, tag="den")
                    ex = attn.tile([128, 2 * W], F32, tag="ex")
                    for h in range(2):
                        src = ps_s[h][:]
                        if use_mask:
                            sm = attn.tile([128, W], F32, tag="sm")
                            nc.vector.tensor_add(sm[:], src, mb[:])
                            src = sm[:]
                        nc.scalar.activation(
                            ex[:, h * W:(h + 1) * W], src, FP.Exp,
                            accum_out=den[:, h:h + 1],
                        )
                    rec = attn.tile([128, 2], F32, tag="rec")
                    nc.vector.reciprocal(rec[:], den[:])
                    # normalize, then transpose each head's probs on the PE
                    pr = attn.tile([128, 2 * W], adt, tag="pr")
                    for h in range(2):
                        nc.vector.tensor_scalar_mul(
                            pr[:, h * W:(h + 1) * W], ex[:, h * W:(h + 1) * W],
                            rec[:, h:h + 1],
                        )
                    ps_t = ppt.tile([128, 2 * W], adt, tag="pt")
                    for h in range(2):
                        nc.tensor.matmul(
                            ps_t[:, h * W:(h + 1) * W], pr[:, h * W:(h + 1) * W],
                            ident[:], is_transpose=True,
                            skip_group_check=(h == 1),
                        )
                    pts = attn.tile([128, 2 * W], adt, tag="pts")
                    nc.vector.tensor_copy(pts[:], ps_t[:])
                    # PV: ctxT pair [128 (2 heads x 64 dims), 128 tokens]
                    ps_c = pcx.tile([128, W], F32, tag="cx")
                    for h in range(2):
                        hd0 = (2 * g + h) * HD
                        nc.tensor.matmul(
                            ps_c[h * 64:(h + 1) * 64, :],
                            vN[cc][:, hd0:hd0 + HD],
                            pts[:, h * W:(h + 1) * W],
                            start=True, stop=True,
                            tile_position=(0, h * 64),
                            skip_group_check=(h == 1),
                        )
                    if wsplit:
                        nc.scalar.activation(cxh[g][:, ts], ps_c[:], FP.Copy)
                        nc.vector.tensor_sub(cxl[g][:, ts], ps_c[:], cxh[g][:, ts])
                    else:
                        nc.vector.tensor_copy(cxh[g][:, ts], ps_c[:])  # casts to wdt

            # -- output projection + residual + LayerNorm --
            if wsplit:
                opasses = [(cxh, "wo"), (cxh, "wo_lo"), (cxl, "wo")]
            else:
                opasses = [(cxh, "wo")]
            for tt in range(CPB):
                r0 = t0 + tt * 128
                xr = outp.tile([128, H], F32, tag="xr")
                nc.sync.dma_start(xr[:], xres[r0:r0 + 128, :])
                hsb = outp.tile([128, H], F32, tag="hsb")
                for nhalf in range(2):
                    n0 = nhalf * 384
                    ps = pproj.tile([128, 384], F32, tag="proj")
                    mms = [(cx, wkey, gi) for cx, wkey in opasses for gi in range(NG)]
                    for i, (cx, wkey, gi) in enumerate(mms):
                        nc.tensor.matmul(
                            ps[:],
                            cx[gi][:, tt * 128:(tt + 1) * 128],
                            w_sb[wkey][gi][:, n0:n0 + 384],
                            start=(i == 0), stop=(i == len(mms) - 1),
                        )
                    nc.vector.tensor_add(hsb[:, n0:n0 + 384], ps[:], xr[:, n0:n0 + 384])

                # LayerNorm over the free dim (H)
                s1 = outp.tile([128, 1], F32, tag="s1")
                nc.vector.reduce_sum(s1[:], hsb[:], axis=AX.X)
                mu = outp.tile([128, 1], F32, tag="mu")
                nc.vector.tensor_scalar_mul(mu[:], s1[:], 1.0 / H)
                xc = outp.tile([128, H], F32, tag="xc")
                nc.vector.tensor_scalar(xc[:], hsb[:], mu[:], None, op0=OP.subtract)
                # var = mean(xc^2) + EPS on DVE (tensor_tensor_reduce crashes
                # on HW; ACT Square risks an Exp<->Square table-set switch)
                sq = outp.tile([128, H], F32, tag="sq")
                nc.vector.tensor_mul(sq[:], xc[:], xc[:])
                s2 = outp.tile([128, 1], F32, tag="s2")
                nc.vector.reduce_sum(s2[:], sq[:], axis=AX.X)
                var1 = outp.tile([128, 1], F32, tag="var1")
                nc.vector.tensor_scalar(var1[:], s2[:], 1.0 / H, EPS, op0=OP.mult, op1=OP.add)
                # rstd = 1/sqrt(var): bit-trick seed + 3 Newton steps (on DVE,
                # avoiding the ACT sqrt table-set switch and its poor ULP)
                rstd = outp.tile([128, 1], F32, tag="rstd")
                t1 = outp.tile([128, 1], F32, tag="t1n")
                ri = rstd[:].bitcast(mybir.dt.int32)
                nc.vector.tensor_scalar(
                    ri, var1[:].bitcast(mybir.dt.int32), 1, None,
                    op0=OP.logical_shift_right,
                )
                nc.vector.tensor_scalar(ri, ri, -1, 0x5F3759DF, op0=OP.mult, op1=OP.add)
                for _ in range(3):
                    nc.vector.tensor_mul(t1[:], rstd[:], rstd[:])
                    nc.vector.tensor_mul(t1[:], t1[:], var1[:])
                    nc.vector.tensor_scalar(t1[:], t1[:], -0.5, 1.5, op0=OP.mult, op1=OP.add)
                    nc.vector.tensor_mul(rstd[:], rstd[:], t1[:])
                ot = outp.tile([128, H], F32, tag="ot")
                nc.vector.tensor_scalar_mul(ot[:], xc[:], rstd[:])
                if use_ln_affine:
                    nc.vector.tensor_mul(ot[:], ot[:], gmb_sb[:])
                    nc.vector.tensor_add(ot[:], ot[:], btb_sb[:])
                nc.sync.dma_start(out[r0:r0 + 128, :], ot[:])

    nc.compile()
    return nc, names


# ---------------------------------------------------------------------------
# host-side wrapper
# ---------------------------------------------------------------------------

_CACHE = {}


def _get_program(mode, use_mask, use_qbias, use_kbias, use_vbias, use_ln_affine, reps=1):
    key = (mode, use_mask, use_qbias, use_kbias, use_vbias, use_ln_affine, reps)
    if key not in _CACHE:
        _CACHE[key] = _build(*key[:-1], reps=reps)
    return _CACHE[key]


def _prep_inputs(inputs, mode):
    """Host preprocessing -> per-core in_maps + program flags."""
    hs = np.ascontiguousarray(np.asarray(inputs["hidden_states"], dtype=np.float32))
    mask = np.asarray(inputs["attention_mask"], dtype=np.float32)
    Wq = np.asarray(inputs["Wq"], np.float32); bq = np.asarray(inputs["bq"], np.float32)
    Wk = np.asarray(inputs["Wk"], np.float32); bk = np.asarray(inputs["bk"], np.float32)
    Wv = np.asarray(inputs["Wv"], np.float32); bv = np.asarray(inputs["bv"], np.float32)
    Wo = np.asarray(inputs["Wo"], np.float32); bo = np.asarray(inputs["bo"], np.float32)
    gm = np.asarray(inputs["ln_gamma"], np.float32)
    bt = np.asarray(inputs["ln_beta"], np.float32)

    cfg = MODES[mode]
    xsplit, wsplit = cfg["xsplit"], cfg["wsplit"]
    npdt = {F32: np.float32, BF16: ml_dtypes.bfloat16, F16: np.float16}[cfg["dt"]]
    use_mask = not np.all(mask == 1.0)
    use_qbias = bool(np.any(bq)); use_kbias = bool(np.any(bk))
    use_vbias = bool(np.any(bv))
    use_ln_affine = bool(np.any(gm != 1.0) or np.any(bt))

    x = hs.reshape(B * S, H)
    xres_full = x + bo[None, :] if np.any(bo) else x

    def wpack(w):
        wh = w.astype(npdt)
        d = {"hi": np.ascontiguousarray(wh)}
        if wsplit:
            d["lo"] = np.ascontiguousarray((w - wh.astype(np.float32)).astype(npdt))
        return d

    wq, wk, wv, wo = wpack(Wq), wpack(Wk), wpack(Wv), wpack(Wo)

    if use_mask:
        # per-core diagonal [W,W] blocks of the mask -> additive bias
        m4 = mask.reshape(B, C, W, C, W)
        idx = np.arange(C)
        mblk = m4[:, idx, :, idx, :]                 # [C,B,W,W]
        mblk = np.transpose(mblk, (1, 0, 2, 3))      # [B,C,W,W]
        bias_blocks = ((1.0 - mblk) * NEG).astype(np.float32).reshape(B * C, W, W)

    in_maps = []
    for c in range(NCORES):
        sl = x[c * TPC:(c + 1) * TPC]                # [TPC, H]
        m = {}
        xh = sl.astype(npdt)
        m["xt_hi"] = np.ascontiguousarray(xh.T)
        if xsplit:
            m["xt_lo"] = np.ascontiguousarray((sl - xh.astype(np.float32)).astype(npdt).T)
        m["xres"] = np.ascontiguousarray(xres_full[c * TPC:(c + 1) * TPC])
        for wn, d in (("wq", wq), ("wk", wk), ("wv", wv), ("wo", wo)):
            m[wn + "_hi"] = d["hi"]
            if wsplit:
                m[wn + "_lo"] = d["lo"]
        if use_qbias:
            m["bq"] = np.ascontiguousarray((bq / 8.0).reshape(NG, 128).T)
        if use_kbias:
            m["bk"] = np.ascontiguousarray(bk.reshape(NG, 128).T)
        if use_vbias:
            m["bvb"] = np.ascontiguousarray(np.broadcast_to(bv, (128, H)))
        if use_ln_affine:
            m["gmb"] = np.ascontiguousarray(np.broadcast_to(gm, (128, H)))
            m["btb"] = np.ascontiguousarray(np.broadcast_to(bt, (128, H)))
        if use_mask:
            m["mbias"] = np.ascontiguousarray(bias_blocks[c * CPC:(c + 1) * CPC])
        in_maps.append(m)

    flags = (use_mask, use_qbias, use_kbias, use_vbias, use_ln_affine)
    return in_maps, flags


def run(inputs, mode=None, trace=False, reps=1):
    """Run the kernel; returns (output [B,S,H] f32, BassKernelResults)."""
    mode = mode or MODE
    in_maps, flags = _prep_inputs(inputs, mode)
    nc, names = _get_program(mode, *flags, reps=reps)
    in_maps = [{k: v for k, v in m.items() if k in names} for m in in_maps]
    res = run_bass_kernel_spmd(nc, in_maps, list(range(NCORES)), trace=trace)
    outs = [res.results[c]["out"] for c in range(NCORES)]
    full = np.concatenate(outs, axis=0).reshape(B, S, H).astype(np.float32)
    return full, res


def kernel(**inputs):
    out, _ = run(inputs)
    return out



# revision 30
# speedup vs baseline: 1.0908x; 1.0908x over previous
"""Trainium2 Bass kernel for nn_CanineAttention (block-diagonal local attention).

Reference computation (per batch b):
  q/k/v = x @ W{q,k,v} + b{q,k,v}            x: [B,S,H]=[4,4096,768]
  per chunk of W=128 tokens, per head (NH=12, HD=64):
    scores = q k^T / 8 + (1-mask_diag)*(-1e4); probs = softmax(scores)
    ctx = probs @ v
  out = LayerNorm(ctx @ Wo + bo + x) * gamma + beta

Sharding: tokens (B*S = 16384) are split contiguously across 8 cores
(2048 tokens = 16 chunks per core; chunk boundaries align), fully
data-parallel, no collectives.

Design (v2, all matmuls single-pass fp16; rel err ~2e-5 vs gate 2e-2):
  - x is consumed TRANSPOSED ([H, tokens], host pre-transposes); Q/K
    projections produce qT/kT in [h_out, t] layout, V in natural [t, h_out].
  - Attention computes scores TRANSPOSED (lhsT=kT, rhs=qT -> [k,q]) so the
    exp'd probs feed P@V directly as lhsT (contraction over k on partitions)
    with no probability transpose.
  - The softmax denominator comes free from PV via an ones-column appended
    per head to the V weights (Wv packed [H, 12*65]; bias tile installs the
    1.0 column), giving [q,65] per head: cols 0-63 ctx, col 64 = den.
    Normalization rides the PSUM->SBUF copy (tensor_scalar_mul by 1/den).
  - ctx [q, hd] is transposed per head on the PE (fp16, half-width) into
    ctxT for the output projection.
  - LayerNorm uses the E[h^2]-E[h]^2 identity: sum(h) comes free from an
    extra Wo column (row-sums) plus host-precomputed residual row-sums;
    sum(h^2) via one ACT Square pass with accum (Exp and Square live in the
    same ACT table set -> no table reloads); rsqrt via bit-trick + Newton
    on DVE (ACT Rsqrt has known accuracy issues).
  - Engine balance: PE is the bottleneck (~36us/block of 4x512 tokens);
    elementwise work is spread over ACT (exp, q/k scale-copies, Square),
    DVE (normalize, LN tail) and Pool (V bias add, residual add, ctxT
    copies) so none exceeds ~half of PE time.
"""

import numpy as np
import ml_dtypes
from contextlib import ExitStack

import concourse.bass as bass
import concourse.tile as tile
from concourse import bacc, mybir
from concourse.bass_utils import run_bass_kernel_spmd
from concourse.masks import make_identity

# ---------------- problem constants (hardcoded per contract) ----------------
B, S, H, NH, W = 4, 4096, 768, 12, 128
HD = H // NH            # 64
C = S // W              # 32 chunks
NEG = -10000.0
EPS = 1e-12

NCORES = 8
TPC = B * S // NCORES   # 2048 tokens per core
CPC = TPC // W          # 16 chunks per core
BLK = 512               # tokens per processing block
NBLK = TPC // BLK       # 4 blocks
CPB = BLK // W          # 4 chunks per block
NG = H // 128           # 6 partition-chunks over H
HDP = HD + 1            # 65: head dim + ones column in packed V
VW = NH * HDP           # 780: packed V width
OW = H + 1              # 769: Wo width + row-sum column

F32 = mybir.dt.float32
F16 = mybir.dt.float16
FP = mybir.ActivationFunctionType
OP = mybir.AluOpType
AX = mybir.AxisListType

MODE = "fp16"
DIVIDE = False  # tensor_scalar(op=divide) fails the walrus ISA check on TRN2


# ---------------------------------------------------------------------------
# kernel builder
# ---------------------------------------------------------------------------

def _build(use_mask, use_qbias, use_kbias, use_ln_affine, reps=1):
    """Build + compile the SPMD single-core program. Returns (nc, input_names).

    reps>1 repeats the whole computation (idempotent output writes) so HW
    kernel time can be measured as the slope over reps."""
    nc = bacc.Bacc(
        "TRN2", target_bir_lowering=False, debug=False,
        enable_asserts=False, num_devices=NCORES,
    )

    # ---------------- DRAM I/O ----------------
    names = []

    def dram_in(name, shape, dt):
        names.append(name)
        return nc.dram_tensor(name, shape, dt, kind="ExternalInput").ap()

    xt = dram_in("xt", [H, TPC], F16)                  # x^T fp16
    xres = dram_in("xres", [TPC, H], F16)              # x + bo (residual input)
    xsum = dram_in("xsum", [128, CPC], F32)            # per-tt row sums of xres
    wq_d = dram_in("wq", [H, H], F16)
    wk_d = dram_in("wk", [H, H], F16)
    wv_d = dram_in("wv", [H, VW], F16)                 # packed V + ones cols
    wo_d = dram_in("wo", [H, OW], F16)                 # Wo + row-sum col
    bvb = dram_in("bvb", [128, VW], F32)               # bv bcast + 1.0 ones cols
    bq = dram_in("bq", [128, 2 * NG], F32) if use_qbias else None  # (bq/8) masked per head
    bk = dram_in("bk", [128, NG], F32) if use_kbias else None
    gmb = dram_in("gmb", [128, H], F32) if use_ln_affine else None
    btb = dram_in("btb", [128, H], F32) if use_ln_affine else None
    mbias = dram_in("mbias", [CPC, W, W], F32) if use_mask else None  # transposed [k,q]
    out = nc.dram_tensor("out", [TPC, H], F32, kind="ExternalOutput").ap()

    with tile.TileContext(nc) as tc, ExitStack() as ctx:
        const = ctx.enter_context(tc.tile_pool(name="const", bufs=1))
        xp = ctx.enter_context(tc.tile_pool(name="xp", bufs=2))
        qkv = ctx.enter_context(tc.tile_pool(name="qkv", bufs=2))
        attn = ctx.enter_context(tc.tile_pool(name="attn", bufs=4))
        ctxp = ctx.enter_context(tc.tile_pool(name="ctxp", bufs=2))
        outp = ctx.enter_context(tc.tile_pool(name="outp", bufs=2))
        # PSUM is 8 banks; slots are bank-granular:
        #   pproj 2 + scores 2 + ps_c 2 + ps_t 2 = 8
        pproj = ctx.enter_context(tc.tile_pool(name="pproj", bufs=2, space="PSUM"))
        psc = ctx.enter_context(tc.tile_pool(name="psc", bufs=2, space="PSUM"))
        pcx = ctx.enter_context(tc.tile_pool(name="pcx", bufs=2, space="PSUM"))

        # ---------------- constants (tiles; DMAs emitted below in an order
        # that lets block-0 compute start as early as possible) ----------
        ident = const.tile([128, 128], F16, tag="ident")
        make_identity(nc, ident)

        def alloc_w(width, tag):
            return [const.tile([128, width], F16, tag=f"{tag}{g}", name=f"{tag}{g}")
                    for g in range(NG)]

        def dma_w(ts, ap_d):
            for g in range(NG):
                nc.sync.dma_start(ts[g][:], ap_d[g * 128:(g + 1) * 128, :])

        wq_sb = alloc_w(H, "wq")
        wk_sb = alloc_w(H, "wk")
        wv_sb = alloc_w(VW, "wv")
        wo_sb = alloc_w(OW, "wo")

        bvb_sb = const.tile([128, VW], F32, tag="bvb")
        xsum_sb = const.tile([128, CPC], F32, tag="xsum")

        # per-head row-selector scales for the zero-padded q tiles:
        # col h (h=0,1): 0.125 on that head-half's partitions, 0 elsewhere
        selq = const.tile([128, 2], F32, tag="selq")
        nc.vector.memset(selq[0:64, 0:1], 0.125)
        nc.vector.memset(selq[64:128, 0:1], 0.0)
        nc.vector.memset(selq[0:64, 1:2], 0.0)
        nc.vector.memset(selq[64:128, 1:2], 0.125)

        bq_sb = bk_sb = gmb_sb = btb_sb = None
        if use_qbias:
            bq_sb = const.tile([128, 2 * NG], F32, tag="bq")
            nc.sync.dma_start(bq_sb[:], bq)
        if use_kbias:
            bk_sb = const.tile([128, NG], F32, tag="bk")
            nc.sync.dma_start(bk_sb[:], bk)
        if use_ln_affine:
            gmb_sb = const.tile([128, H], F32, tag="gmb")
            nc.sync.dma_start(gmb_sb[:], gmb)
            btb_sb = const.tile([128, H], F32, tag="btb")
            nc.sync.dma_start(btb_sb[:], btb)

        # ---------------- per token-block ----------------
        HVW = VW // 2   # 390
        LAG = 3

        def emit_xt_load(blk):
            t0 = blk * BLK
            xth = [xp.tile([128, BLK], F16, tag=f"xth{g}", name=f"xth{g}")
                   for g in range(NG)]
            for g in range(NG):
                nc.sync.dma_start(xth[g][:], xt[g * 128:(g + 1) * 128, t0:t0 + BLK])
            return xth

        def emit_xres_load(blk):
            t0 = blk * BLK
            xrs = []
            for tt in range(CPB):
                xr = outp.tile([128, H], F16, tag=f"xr{tt}", name=f"xr{tt}")
                r0 = t0 + tt * 128
                nc.sync.dma_start(xr[:], xres[r0:r0 + 128, :])
                xrs.append(xr)
            return xrs

        def emit_qproj(xth, go):
            """Q projection for out-group go -> two zero-padded per-head
            tiles [128, BLK] (head rows carry q/8, other rows zero) so the
            score matmuls can contract over the full 128 partitions."""
            ps = pproj.tile([128, BLK], F32, tag="proj")
            for gi in range(NG):
                nc.tensor.matmul(
                    ps[:], wq_sb[gi][:, go * 128:(go + 1) * 128], xth[gi][:],
                    start=(gi == 0), stop=(gi == NG - 1),
                )
            qz = []
            for h in range(2):
                sb = qkv.tile([128, BLK], F16, tag=f"qz{2 * go + h}",
                              name=f"qz{2 * go + h}")
                if use_qbias:
                    nc.scalar.activation(sb[:], ps[:], FP.Identity,
                                         bias=bq_sb[:, 2 * go + h:2 * go + h + 1],
                                         scale=selq[:, h:h + 1])
                else:
                    nc.scalar.activation(sb[:], ps[:], FP.Identity,
                                         scale=selq[:, h:h + 1])
                qz.append(sb)
            return qz

        def emit_kproj(xth, go):
            ps = pproj.tile([128, BLK], F32, tag="proj")
            for gi in range(NG):
                nc.tensor.matmul(
                    ps[:], wk_sb[gi][:, go * 128:(go + 1) * 128], xth[gi][:],
                    start=(gi == 0), stop=(gi == NG - 1),
                )
            sb = qkv.tile([128, BLK], F16, tag=f"kT{go}", name=f"kT{go}")
            if use_kbias:
                nc.scalar.activation(sb[:], ps[:], FP.Identity,
                                     bias=bk_sb[:, go:go + 1])
            else:
                nc.scalar.activation(sb[:], ps[:], FP.Copy)
            return sb

        def emit_vproj(xth, tt):
            vt = qkv.tile([128, VW], F16, tag=f"v{tt}", name=f"v{tt}")
            for nhalf in range(2):
                n0 = nhalf * HVW
                ps = pproj.tile([128, BLK], F32, tag="proj")
                for gi in range(NG):
                    nc.tensor.matmul(
                        ps[:, 0:HVW],
                        xth[gi][:, tt * 128:(tt + 1) * 128],
                        wv_sb[gi][:, n0:n0 + HVW],
                        start=(gi == 0), stop=(gi == NG - 1),
                    )
                # installs bv and the per-head 1.0 ones-columns
                # (GPSIMD/Pool cannot read PSUM -> DVE; V phase has DVE slack)
                nc.vector.tensor_add(vt[:, n0:n0 + HVW], ps[:, 0:HVW],
                                     bvb_sb[:, n0:n0 + HVW])
            return vt

        def emit_scores(blk, qz, kT, cc, g):
            ts = slice(cc * 128, (cc + 1) * 128)
            chunk_idx = blk * CPB + cc
            # both heads' scoresT[k,q] in one PSUM tile: full-K matmuls
            # (zero-padded q) are not quadrant-concurrent, so same-bank
            # column-range writes are safe.
            ps_s = psc.tile([128, 2 * W], F32, tag="sc", name="ps_s")
            for h in range(2):
                nc.tensor.matmul(
                    ps_s[:, h * W:(h + 1) * W],
                    kT[g][:, ts],
                    qz[2 * g + h][:, ts],
                    start=True, stop=True,
                    skip_group_check=(h == 1),
                )
            ext = attn.tile([128, 2 * W], F16, tag="ext")
            if use_mask:
                mb = attn.tile([128, W], F32, tag="mb")
                nc.sync.dma_start(mb[:], mbias[chunk_idx])
                sm = attn.tile([128, 2 * W], F32, tag="sm")
                for h in range(2):
                    nc.vector.tensor_add(sm[:, h * W:(h + 1) * W],
                                         ps_s[:, h * W:(h + 1) * W], mb[:])
                nc.scalar.activation(ext[:], sm[:], FP.Exp)
            else:
                nc.scalar.activation(ext[:], ps_s[:], FP.Exp)
            return (cc, g, ext)

        def emit_pv(vN, ps_ts, cxhs, state):
            cc, g, ext = state
            # PV + den: out [q, 65] per head (col 64 = sum_k exp)
            ps_c = pcx.tile([128, 2 * HDP], F32, tag="cx", name="ps_c")
            for h in range(2):
                nc.tensor.matmul(
                    ps_c[:, h * HDP:(h + 1) * HDP],
                    ext[:, h * W:(h + 1) * W],
                    vN[cc][:, (2 * g + h) * HDP:(2 * g + h + 1) * HDP],
                    start=True, stop=True,
                    skip_group_check=(h == 1),
                )
            ctxn = attn.tile([128, 2 * HD], F16, tag="ctxn")
            if DIVIDE:
                for h in range(2):
                    nc.vector.tensor_scalar(
                        ctxn[:, h * HD:(h + 1) * HD],
                        ps_c[:, h * HDP:h * HDP + HD],
                        ps_c[:, h * HDP + HD:h * HDP + HD + 1],
                        None, op0=OP.divide,
                    )
            else:
                rec = attn.tile([128, 2], F32, tag="rec")
                nc.vector.reciprocal(rec[:], ps_c[:, HD:2 * HDP:HDP])
                for h in range(2):
                    nc.vector.tensor_scalar_mul(
                        ctxn[:, h * HD:(h + 1) * HD],
                        ps_c[:, h * HDP:h * HDP + HD],
                        rec[:, h:h + 1],
                    )
            # transpose ctx -> ctxT [hd, q]; all 6 pair-groups of the chunk
            # accumulate in one [128, H] fp16 PSUM tile (fits one bank), so
            # a single wide DVE copy moves the chunk's ctxT to SBUF.
            ps_t = ps_ts[cc]
            for h in range(2):
                nc.tensor.matmul(
                    ps_t[h * HD:(h + 1) * HD, g * W:(g + 1) * W],
                    ctxn[:, h * HD:(h + 1) * HD],
                    ident[:], is_transpose=True,
                    skip_group_check=(h == 1),
                )
            if g == NG - 1:
                nc.vector.tensor_copy(cxhs[cc][:], ps_t[:])

        def emit_oproj(blk, cxhs, xrs, tt):
            """O-projection matmuls + residual add for chunk tt of block blk
            (called during the NEXT block's projection phase)."""
            tti = blk * CPB + tt
            xr = xrs[tt]
            hsb = outp.tile([128, H], F32, tag=f"hsb{tt}", name=f"hsb{tt}")
            sh = outp.tile([128, 1], F32, tag=f"sh{tt}", name=f"sh{tt}")
            for nhalf in range(2):
                n0 = nhalf * 384
                n1 = 385 if nhalf == 1 else 384
                ps = pproj.tile([128, BLK], F32, tag="proj")
                for gi in range(NG):
                    nc.tensor.matmul(
                        ps[:, 0:n1],
                        cxhs[tt][:, gi * 128:(gi + 1) * 128],
                        wo_sb[gi][:, n0:n0 + n1],
                        start=(gi == 0), stop=(gi == NG - 1),
                    )
                nc.vector.tensor_add(hsb[:, n0:n0 + 384], ps[:, 0:384],
                                     xr[:, n0:n0 + 384])
                if nhalf == 1:
                    # sum_h hsb = (ctx@Wo row-sum col) + host residual sums
                    nc.vector.tensor_add(sh[:], ps[:, 384:385],
                                         xsum_sb[:, tti:tti + 1])
            # -- LayerNorm over the free dim (H) --
            r0 = blk * BLK + tt * 128
            s2 = outp.tile([128, 1], F32, tag="s2")
            sqd = outp.tile([128, H], F16, tag="sqd")   # discarded
            nc.scalar.activation(sqd[:], hsb[:], FP.Square, accum_out=s2[:])
            mu = outp.tile([128, 1], F32, tag="mu")
            nc.vector.tensor_scalar_mul(mu[:], sh[:], 1.0 / H)
            t0_ = outp.tile([128, 1], F32, tag="t0n")
            nc.vector.tensor_mul(t0_[:], mu[:], sh[:])
            t1_ = outp.tile([128, 1], F32, tag="t1n")
            nc.vector.tensor_sub(t1_[:], s2[:], t0_[:])
            var1 = outp.tile([128, 1], F32, tag="var1")
            nc.vector.tensor_scalar(var1[:], t1_[:], 1.0 / H, EPS,
                                    op0=OP.mult, op1=OP.add)
            # rstd = 1/sqrt(var): bit-trick seed + 3 Newton steps (on DVE,
            # avoiding the ACT rsqrt accuracy problem)
            rstd = outp.tile([128, 1], F32, tag="rstd")
            tn = outp.tile([128, 1], F32, tag="tn")
            ri = rstd[:].bitcast(mybir.dt.int32)
            nc.vector.tensor_scalar(
                ri, var1[:].bitcast(mybir.dt.int32), 1, None,
                op0=OP.logical_shift_right,
            )
            nc.vector.tensor_scalar(ri, ri, -1, 0x5F3759DF, op0=OP.mult, op1=OP.add)
            for _ in range(3):
                nc.vector.tensor_mul(tn[:], rstd[:], rstd[:])
                nc.vector.tensor_mul(tn[:], tn[:], var1[:])
                nc.vector.tensor_scalar(tn[:], tn[:], -0.5, 1.5, op0=OP.mult, op1=OP.add)
                nc.vector.tensor_mul(rstd[:], rstd[:], tn[:])
            ot = outp.tile([128, H], F32, tag="ot")
            nc.vector.tensor_scalar(ot[:], hsb[:], mu[:], rstd[:],
                                    op0=OP.subtract, op1=OP.mult)
            if use_ln_affine:
                nc.vector.tensor_mul(ot[:], ot[:], gmb_sb[:])
                nc.vector.tensor_add(ot[:], ot[:], btb_sb[:])
            nc.sync.dma_start(out[r0:r0 + 128, :], ot[:])

        def emit_block(blk, prev, pre=None):
            """Emit one block: projection phase (with the PREVIOUS block's
            O-projections/LayerNorm interleaved into the dense matmul
            stream), then the attention phase."""
            if pre is not None:
                xth, xrs = pre
            else:
                xth = emit_xt_load(blk)
                xrs = emit_xres_load(blk)
            qz = [None] * (2 * NG)
            kT = [None] * NG
            # Q then K projections; previous block's 4 O-proj chunks are
            # spread through the 12 projection groups.
            oslots = {2: 0, 5: 1, 8: 2, 11: 3} if prev is not None else {}
            step = 0
            for go in range(NG):
                q2 = emit_qproj(xth, go)
                qz[2 * go], qz[2 * go + 1] = q2
                if step in oslots:
                    pcxh, pxrs, pblk = prev
                    emit_oproj(pblk, pcxh, pxrs, oslots[step])
                step += 1
            for go in range(NG):
                kT[go] = emit_kproj(xth, go)
                if step in oslots:
                    pcxh, pxrs, pblk = prev
                    emit_oproj(pblk, pcxh, pxrs, oslots[step])
                step += 1
            vN = [emit_vproj(xth, tt) for tt in range(CPB)]

            cxhs = [ctxp.tile([128, H], F16, tag=f"cxh{cc}", name=f"cxh{cc}")
                    for cc in range(CPB)]
            ps_ts = [pcx.tile([128, H], F16, tag="pt", name=f"ps_t{cc}")
                     for cc in range(CPB)]
            stages = []
            for cc in range(CPB):
                for g in range(NG):
                    stages.append(emit_scores(blk, qz, kT, cc, g))
                    if len(stages) > LAG:
                        emit_pv(vN, ps_ts, cxhs, stages.pop(0))
            while stages:
                emit_pv(vN, ps_ts, cxhs, stages.pop(0))
            return (cxhs, xrs, blk)

        # const DMAs, ordered so block-0's first matmuls start ~5us in:
        # wq -> (block-0 x^T) -> wk -> wv -> (block-0 xres) -> wo -> rest
        pre0 = None
        dma_w(wq_sb, wq_d)
        if reps == 1:
            pre_xth = emit_xt_load(0)
        dma_w(wk_sb, wk_d)
        dma_w(wv_sb, wv_d)
        if reps == 1:
            pre0 = (pre_xth, emit_xres_load(0))
        dma_w(wo_sb, wo_d)
        nc.sync.dma_start(bvb_sb[:], bvb)
        nc.sync.dma_start(xsum_sb[:], xsum)

        import contextlib
        rep_cm = tc.For_i(0, reps, 1) if reps > 1 else contextlib.nullcontext()
        with rep_cm:
            prev = None
            for blk in range(NBLK):
                prev = emit_block(blk, prev, pre0 if blk == 0 else None)
                pre0 = None
            # flush the last block's output projections
            pcxh, pxrs, pblk = prev
            for tt in range(CPB):
                emit_oproj(pblk, pcxh, pxrs, tt)

    nc.compile()
    return nc, names


# ---------------------------------------------------------------------------
# host-side wrapper
# ---------------------------------------------------------------------------

_CACHE = {}


def _get_program(use_mask, use_qbias, use_kbias, use_ln_affine, reps=1):
    key = (use_mask, use_qbias, use_kbias, use_ln_affine, reps)
    if key not in _CACHE:
        _CACHE[key] = _build(*key[:-1], reps=reps)
    return _CACHE[key]


def _prep_inputs(inputs):
    """Host preprocessing -> per-core in_maps + program flags."""
    hs = np.ascontiguousarray(np.asarray(inputs["hidden_states"], dtype=np.float32))
    mask = np.asarray(inputs["attention_mask"], dtype=np.float32)
    Wq = np.asarray(inputs["Wq"], np.float32); bq = np.asarray(inputs["bq"], np.float32)
    Wk = np.asarray(inputs["Wk"], np.float32); bk = np.asarray(inputs["bk"], np.float32)
    Wv = np.asarray(inputs["Wv"], np.float32); bv = np.asarray(inputs["bv"], np.float32)
    Wo = np.asarray(inputs["Wo"], np.float32); bo = np.asarray(inputs["bo"], np.float32)
    gm = np.asarray(inputs["ln_gamma"], np.float32)
    bt = np.asarray(inputs["ln_beta"], np.float32)

    f16 = np.float16
    use_mask = not np.all(mask == 1.0)
    use_qbias = bool(np.any(bq)); use_kbias = bool(np.any(bk))
    use_ln_affine = bool(np.any(gm != 1.0) or np.any(bt))

    x = hs.reshape(B * S, H)
    xres_full = x + bo[None, :] if np.any(bo) else x

    # packed V weights [H, 12*65] with a zero (-> bias 1.0) ones-column per head
    wv_p = np.zeros((H, VW), np.float32)
    bvb_full = np.zeros((VW,), np.float32)
    for h in range(NH):
        wv_p[:, h * HDP:h * HDP + HD] = Wv[:, h * HD:(h + 1) * HD]
        bvb_full[h * HDP:h * HDP + HD] = bv[h * HD:(h + 1) * HD]
        bvb_full[h * HDP + HD] = 1.0

    # Wo + row-sums column (for the LayerNorm mean)
    wo_p = np.concatenate([Wo, Wo.sum(axis=1, keepdims=True)], axis=1)

    if use_mask:
        # per-core diagonal [W,W] blocks of the mask -> additive bias,
        # TRANSPOSED to [k,q] to match the transposed score layout
        m4 = mask.reshape(B, C, W, C, W)
        idx = np.arange(C)
        mblk = m4[:, idx, :, idx, :]                 # [C,B,W,W]
        mblk = np.transpose(mblk, (1, 0, 3, 2))      # [B,C,Wk,Wq]
        bias_blocks = ((1.0 - mblk) * NEG).astype(np.float32).reshape(B * C, W, W)

    wq16 = np.ascontiguousarray(Wq.astype(f16))
    wk16 = np.ascontiguousarray(Wk.astype(f16))
    wv16 = np.ascontiguousarray(wv_p.astype(f16))
    wo16 = np.ascontiguousarray(wo_p.astype(f16))
    bvb_b = np.ascontiguousarray(np.broadcast_to(bvb_full, (128, VW)))

    in_maps = []
    for c in range(NCORES):
        sl = x[c * TPC:(c + 1) * TPC]                # [TPC, H]
        xres_c = xres_full[c * TPC:(c + 1) * TPC].astype(f16)
        m = {
            "xt": np.ascontiguousarray(sl.astype(f16).T),
            "xres": np.ascontiguousarray(xres_c),
            "xsum": np.ascontiguousarray(
                xres_c.astype(np.float64).sum(axis=1).astype(np.float32)
                .reshape(CPC, 128).T),
            "wq": wq16, "wk": wk16, "wv": wv16, "wo": wo16,
            "bvb": bvb_b,
        }
        if use_qbias:
            # masked per padded head-tile: col 2*go+h = bq/8 for out-group go
            # on head-h rows (the zero rows must stay zero)
            bqz = np.zeros((128, 2 * NG), np.float32)
            bqc = (bq / 8.0).reshape(NG, 128)
            for go in range(NG):
                for h in range(2):
                    rows = slice(h * 64, (h + 1) * 64)
                    bqz[rows, 2 * go + h] = bqc[go][rows]
            m["bq"] = np.ascontiguousarray(bqz)
        if use_kbias:
            m["bk"] = np.ascontiguousarray(bk.reshape(NG, 128).T)
        if use_ln_affine:
            m["gmb"] = np.ascontiguousarray(np.broadcast_to(gm, (128, H)))
            m["btb"] = np.ascontiguousarray(np.broadcast_to(bt, (128, H)))
        if use_mask:
            m["mbias"] = np.ascontiguousarray(bias_blocks[c * CPC:(c + 1) * CPC])
        in_maps.append(m)

    flags = (use_mask, use_qbias, use_kbias, use_ln_affine)
    return in_maps, flags


def run(inputs, mode=None, trace=False, reps=1):
    """Run the kernel; returns (output [B,S,H] f32, BassKernelResults)."""
    in_maps, flags = _prep_inputs(inputs)
    nc, names = _get_program(*flags, reps=reps)
    in_maps = [{k: v for k, v in m.items() if k in names} for m in in_maps]
    res = run_bass_kernel_spmd(nc, in_maps, list(range(NCORES)), trace=trace)
    outs = [res.results[c]["out"] for c in range(NCORES)]
    full = np.concatenate(outs, axis=0).reshape(B, S, H).astype(np.float32)
    return full, res


def kernel(**inputs):
    out, _ = run(inputs)
    return out
